# revision 22
# baseline (speedup 1.0000x reference)
"""DiSA (directional self-attention) Bass kernel for Trainium2, 8 cores.

Factorized algorithm (no [S,S,D] intermediate): with
  w = e^{a+b} * psi(a+b),  psi(x) = exp(C*tanh(x/C) - x),  a=dep, b=head,
approximate psi by a degree-K polynomial P (weighted LS fit on the actual
a+b range).  Taylor expansion P(a+b) = sum_m b^m Q_m(a) makes the softmax
separable; e^b cancels in the ratio:
  attn_res[i,d] = sum_m b_i^m N_m(i) / sum_m b_i^m D_m(i)
  D_m(i) = suffix_{j>i}[ mask_j e^{a_j} Q_m(a_j) ]          (x rep_j for N_m)

Mapping (per core c -> batch b=c//2, d-half h=c%2, 150 planes):
  - series: Horner ladders split across DVE+GpSimd (fp16, flat [128,300]),
    mask folded into e^a via ACT bias.  Slot-major storage (c, s, d).
  - suffix sums over j: PE matmuls, strict-upper-tri / all-ones stationaries;
    moving AP iterates (d outer, s inner) -> d-major PSUM, K=4 -> 375 cols
    fits one PSUM bank (no chunking).
  - sum_m b^m X_m: tensor_tensor_scan (state = b*state + X_m), d-major,
    b-repeat tile with 0 at slot 0 of each d-block; scans split DVE/GpSimd.
  - phase C: each core computes gate pre-activation PARTIALS over its own
    d-half only (Wf1 part early, Wf2 x own-attn after phase B), pairwise
    ReduceScatter(add) gives each core the full pre-act for its own g-half;
    sigmoid-blend in T layout; host concatenates the two cores' outT halves.
    A per-core d-permutation (h=1 swaps halves) keeps the program uniform.
"""

import numpy as np
from math import comb as _comb

B, S, D = 4, 256, 300
C = 5.0
HALF = D // 2          # 150 d-planes per core
K = 4                  # psi polynomial degree
NS = K + 1             # slots per d-plane
WB = HALF * NS         # 750 cols per c-block (slot-major series)
QD = 75                # d-planes per suffix/scan half
QW = QD * NS           # 375 cols per suffix psum (single PSUM bank)

# crow packed consts layout: [ones(256) | b_fc_perm(300) | b_fch(150) |
#                             b1h(150) | b_f(300) | 0.5*rm(256)]
O_ONES, O_BFC, O_BFCH, O_B1H, O_BF, O_HM = 0, 256, 556, 706, 856, 1156
CROW_W = 1412

_CACHE: dict = {}


def _q_coeffs():
    # weighted LS fit of psi on the actual a+b range (Gaussian weight
    # matching the data distribution; measured range ~[-5.6, 4.9])
    lo, hi, sig = -5.8, 5.1, 2.0
    xs = np.linspace(lo, hi, 12001)
    psi = np.exp(C * np.tanh(xs / C) - xs)
    w = np.exp(-xs ** 2 / (2 * sig ** 2)) + 1e-3
    V = np.vander(xs, K + 1, increasing=True) * w[:, None]
    c, *_ = np.linalg.lstsq(V, psi * w, rcond=None)
    return [[float(c[m + j] * _comb(m + j, m)) for j in range(K - m + 1)]
            for m in range(K + 1)]


QC = _q_coeffs()


def _chunks(total, step=128):
    return [(s, min(step, total - s)) for s in range(0, total, step)]


def _build_nc():
    import concourse.bass as bass
    import concourse.tile as tile
    from concourse import bacc, mybir

    F32 = mybir.dt.float32
    F16 = mybir.dt.float16
    AF = mybir.ActivationFunctionType
    OP = mybir.AluOpType

    nc = bacc.Bacc("TRN2", target_bir_lowering=False, debug=False, num_devices=8)

    def din(name, shape, dt=F16):
        return nc.dram_tensor(name, shape, dt, kind="ExternalInput").ap()

    inputsT_d = din("inputsT", [D, S])
    W_fcT_d = din("W_fcT", [D, D])          # cols permuted per core
    W_fcTh_d = din("W_fcTh", [D, HALF])
    W1Th_d = din("W1Th", [D, HALF])         # rows permuted per core
    W2Th_d = din("W2Th", [D, HALF])
    Wf1Tm_d = din("Wf1Tm", [HALF, D])       # rows = my dd half
    Wf2Tm_d = din("Wf2Tm", [HALF, D])
    crow_d = din("consts_row", [1, CROW_W])
    mats_d = din("mats", [128, 384])        # [ident | su_tri | ones]
    maskb_d = din("maskbias", [128, 2], F32)
    bcol_d = din("bfc_col", [D, 1], F32)    # b_fc (permuted) per-partition column
    outT_d = nc.dram_tensor("outT", [HALF, S], F32, kind="ExternalOutput").ap()

    DC = _chunks(D)
    GC = _chunks(D)  # g-chunks for phase C psum

    with tile.TileContext(nc) as tc:
        with (
            tc.tile_pool(name="persist", bufs=1) as pp,
            tc.tile_pool(name="dram", bufs=1, space="DRAM") as dram,
        ):
            # ---- input DMAs: critical tensors first, spread across queues
            inT = [pp.tile([n, S], F16, tag=f"inT{i}", name=f"inT{i}") for i, (o, n) in enumerate(DC)]
            WfcT = [pp.tile([n, D], F16, tag=f"wfc{i}", name=f"wfc{i}") for i, (o, n) in enumerate(DC)]
            WfcTh = [pp.tile([n, HALF], F16, tag=f"wfch{i}", name=f"wfch{i}") for i, (o, n) in enumerate(DC)]
            W1T = [pp.tile([n, HALF], F16, tag=f"w1{i}", name=f"w1_{i}") for i, (o, n) in enumerate(DC)]
            W2T = [pp.tile([n, HALF], F16, tag=f"w2{i}", name=f"w2_{i}") for i, (o, n) in enumerate(DC)]
            Wf1m = [pp.tile([QD, D], F16, tag=f"wg1{i}", name=f"wg1_{i}") for i in range(2)]
            Wf2m = [pp.tile([QD, D], F16, tag=f"wg2{i}", name=f"wg2_{i}") for i in range(2)]
            crow = pp.tile([1, CROW_W], F16)
            mats = pp.tile([128, 384], F16)
            maskb = pp.tile([128, 2], F32)
            bcol = [pp.tile([n, 1], F32, tag=f"bc{i}", name=f"bc{i}") for i, (o, n) in enumerate(DC)]

            for i, (o, n) in enumerate(DC):
                nc.sync.dma_start(inT[i][:], inputsT_d[o : o + n, :])
                nc.scalar.dma_start(WfcT[i][:], W_fcT_d[o : o + n, :])
            nc.sync.dma_start(crow[:], crow_d[:])
            nc.scalar.dma_start(maskb[:], maskb_d[:])
            for i, (o, n) in enumerate(DC):
                nc.gpsimd.dma_start(bcol[i][:], bcol_d[o : o + n, :])
            for i, (o, n) in enumerate(DC):
                nc.sync.dma_start(W1T[i][:], W1Th_d[o : o + n, :])
                nc.scalar.dma_start(W2T[i][:], W2Th_d[o : o + n, :])
                nc.gpsimd.dma_start(WfcTh[i][:], W_fcTh_d[o : o + n, :])
            nc.gpsimd.dma_start(mats[:], mats_d[:])
            for i in range(2):
                nc.gpsimd.dma_start(Wf1m[i][:], Wf1Tm_d[i * QD : (i + 1) * QD, :])
                nc.gpsimd.dma_start(Wf2m[i][:], Wf2Tm_d[i * QD : (i + 1) * QD, :])

            repT = [pp.tile([n, S], F16, tag=f"repT{i}", name=f"repT{i}") for i, (o, n) in enumerate(DC)]
            repTm = [pp.tile([QD, S], F16, tag=f"rtm{i}", name=f"rtm{i}") for i in range(2)]
            rep_nat = pp.tile([128, 2 * HALF], F16)
            dep_nat = pp.tile([128, 2 * HALF], F16)
            head_nat = pp.tile([128, 2 * HALF], F16)
            E_t = pp.tile([128, 2 * HALF], F16)
            SERd = pp.tile([128, 2 * WB], F16)   # slot-major: col = c*WB + s*HALF + d
            SERn = pp.tile([128, 2 * WB], F16)
            bblk = pp.tile([128, 2 * WB], F16)   # d-major: col = c*WB + d*NS + s
            scd = [pp.tile([128, WB], F16, tag=f"scd{i}", name=f"scd{i}") for i in range(2)]
            scn = [pp.tile([128, WB], F16, tag=f"scn{i}", name=f"scn{i}") for i in range(2)]
            attn_nat = pp.tile([128, 2 * HALF], F16)
            a75 = [pp.tile([QD, S], F16, tag=f"a75_{i}", name=f"a75_{i}") for i in range(2)]
            pre16 = [pp.tile([n, S], F16, tag=f"p16{i}", name=f"p16{i}") for i, (o, n) in enumerate(GC)]
            th16 = [pp.tile([QD, S], F16, tag=f"th{i}", name=f"th{i}") for i in range(2)]
            Mb = pp.tile([QD, S], F16)
            rs_in = dram.tile([D, S], F16, tag="rsi", name="rsi")
            rs_out = dram.tile([HALF, S], F16, tag="rso", name="rso")

            # ---------- phase A ----------
            with (
                tc.tile_pool(name="pa_ps", bufs=2, space="PSUM") as pa_ps,
                tc.tile_pool(name="pa_sb", bufs=2) as pa_sb,
            ):
                def elu_from_psum(ps_ap, out_ap, n, bias=None):
                    # elu(x) = min(exp(x) - 1, relu(x)):
                    #   exp on ACT, relu + fused min on DVE (GpSimd can't read
                    #   PSUM).  Optional per-partition bias column folded in.
                    w = ps_ap.shape[1]
                    ex = pa_sb.tile([n, w], F16, tag="elu_e", name="elu_e")
                    nc.scalar.activation(
                        ex[:], ps_ap, AF.Exp,
                        bias=(bias if bias is not None else 0.0),
                    )
                    rl = pa_sb.tile([n, w], F16, tag="elu_r", name="elu_r")
                    if bias is None:
                        nc.vector.tensor_scalar(
                            out=rl[:], in0=ps_ap, scalar1=0.0, scalar2=None, op0=OP.max
                        )
                    else:
                        nc.vector.tensor_scalar(
                            out=rl[:], in0=ps_ap, scalar1=bias, scalar2=0.0,
                            op0=OP.add, op1=OP.max,
                        )
                    nc.vector.scalar_tensor_tensor(
                        out=out_ap, in0=ex[:], scalar=-1.0, in1=rl[:],
                        op0=OP.add, op1=OP.min,
                    )

                for i, (o, n) in enumerate(DC):
                    ps = pa_ps.tile([n, S], F32, tag="paT", name="paT")
                    for k in range(3):
                        nc.tensor.matmul(
                            ps[:], WfcT[k][:, o : o + n], inT[k][:],
                            start=(k == 0), stop=(k == 2),
                        )
                    elu_from_psum(ps[:], repT[i][:], n, bias=bcol[i][:])

                # Mb = broadcast of 0.5*rep_mask row (PE outer product)
                mps = pa_ps.tile([QD, S], F32, tag="mps", name="mps")
                nc.tensor.matmul(
                    mps[:], crow[0:1, O_ONES : O_ONES + QD],
                    crow[0:1, O_HM : O_HM + S],
                    start=True, stop=True,
                )
                nc.scalar.activation(Mb[:], mps[:], AF.Copy)

                # repTm copies (sbuf->sbuf DMA; rows 0:150 of permuted repT)
                nc.sync.dma_start(repTm[0][:], repT[0][0:QD, :])
                nc.sync.dma_start(repTm[1][0 : 128 - QD, :], repT[0][QD:128, :])
                nc.scalar.dma_start(repTm[1][128 - QD : QD, :], repT[1][0:22, :])

                for cc in range(2):
                    so = 128 * cc
                    ps = pa_ps.tile([128, HALF], F32, tag="paN", name="paN")
                    for k in range(3):
                        nc.tensor.matmul(
                            ps[:], inT[k][:, so : so + 128], WfcTh[k][:],
                            start=(k == 0), stop=False,
                        )
                    nc.tensor.matmul(
                        ps[:], crow[0:1, O_ONES : O_ONES + 128],
                        crow[0:1, O_BFCH : O_BFCH + HALF],
                        start=False, stop=True,
                    )
                    elu_from_psum(ps[:], rep_nat[:, cc * HALF : (cc + 1) * HALF], 128)

                for cc in range(2):
                    so = 128 * cc
                    ps = pa_ps.tile([128, HALF], F32, tag="paN", name="paN")
                    for k in range(3):
                        nc.tensor.matmul(
                            ps[:], repT[k][:, so : so + 128], W1T[k][:],
                            start=(k == 0), stop=False,
                        )
                    nc.tensor.matmul(
                        ps[:], crow[0:1, O_ONES : O_ONES + 128],
                        crow[0:1, O_B1H : O_B1H + HALF],
                        start=False, stop=True,
                    )
                    # E = exp(dep + maskbias) straight from psum; dep copy on GpSimd
                    nc.scalar.activation(
                        E_t[:, cc * HALF : (cc + 1) * HALF], ps[:], AF.Exp,
                        bias=maskb[:, cc : cc + 1], scale=1.0,
                    )
                    nc.scalar.activation(
                        dep_nat[:, cc * HALF : (cc + 1) * HALF], ps[:], AF.Copy
                    )

                    ps2 = pa_ps.tile([128, HALF], F32, tag="paN", name="paN")
                    for k in range(3):
                        nc.tensor.matmul(
                            ps2[:], repT[k][:, so : so + 128], W2T[k][:],
                            start=(k == 0), stop=(k == 2),
                        )
                    nc.scalar.activation(
                        head_nat[:, cc * HALF : (cc + 1) * HALF], ps2[:], AF.Copy
                    )

            # ---------- phase B + C ----------
            with (
                tc.tile_pool(name="pb_sb", bufs=2) as pb_sb,
                tc.tile_pool(name="pb_ps", bufs=2, space="PSUM") as pb_ps,
                tc.tile_pool(name="pc_ps", bufs=1, space="PSUM") as pc_ps,
                tc.tile_pool(name="pc_sb", bufs=2) as pc_sb,
            ):
                # phase-C Wf1 partial (early: PE stays warm during ladders)
                pcp = [pc_ps.tile([n, S], F32, tag=f"pcp{i}", name=f"pcp{i}") for i, (o, n) in enumerate(GC)]
                for i, (go, gn) in enumerate(GC):
                    nc.tensor.matmul(
                        pcp[i][:], Wf1m[0][:, go : go + gn], repTm[0][:],
                        start=True, stop=False,
                    )
                    nc.tensor.matmul(
                        pcp[i][:], Wf1m[1][:, go : go + gn], repTm[1][:],
                        start=False, stop=False,
                    )
                    nc.tensor.matmul(
                        pcp[i][:], crow[0:1, O_BF + go : O_BF + go + gn],
                        crow[0:1, O_ONES : O_ONES + S],
                        start=False, stop=False,
                    )

                # b-repeat tile (d-major): slot 0 -> 0, slots 1..K -> head (DVE)
                bb4 = bblk[:].rearrange("p (c d s) -> p c d s", c=2, s=NS)
                h3 = head_nat[:].rearrange("p (c d) -> p c d", c=2).unsqueeze(3)
                for s in range(1, NS):
                    nc.vector.tensor_scalar(
                        out=bb4[:, :, :, s : s + 1], in0=h3,
                        scalar1=0.0, scalar2=None, op0=OP.add,
                    )
                nc.vector.memset(bb4[:, :, :, 0:1], 0.0)

                # Horner ladders -> slot-major series (all DVE; GpSimd lacks
                # TensorScalarPtr).  den finals first (suffix den matmuls
                # start early); num series = den series * rep (fast tt).
                sd4 = SERd[:].rearrange("p (c s d) -> p c s d", c=2, s=NS)
                sn4 = SERn[:].rearrange("p (c s d) -> p c s d", c=2, s=NS)
                E3 = E_t[:].rearrange("p (c d) -> p c d", c=2)
                rep3 = rep_nat[:].rearrange("p (c d) -> p c d", c=2)

                for m in range(K + 1):
                    n = K - m
                    s = K - m
                    if n == 0:
                        nc.vector.tensor_scalar(
                            out=sd4[:, :, s, :], in0=E3,
                            scalar1=QC[m][0], scalar2=None, op0=OP.mult,
                        )
                        continue
                    acc = pb_sb.tile([128, 2 * HALF], F16, tag=f"acc{m}", name=f"acc{m}")
                    nc.vector.tensor_scalar(
                        out=acc[:], in0=dep_nat[:],
                        scalar1=QC[m][n], scalar2=None, op0=OP.mult,
                    )
                    for j in range(n - 1, 0, -1):
                        nc.vector.scalar_tensor_tensor(
                            out=acc[:], in0=acc[:], scalar=QC[m][j], in1=dep_nat[:],
                            op0=OP.add, op1=OP.mult,
                        )
                    acc3 = acc[:].rearrange("p (c d) -> p c d", c=2)
                    nc.vector.scalar_tensor_tensor(
                        out=sd4[:, :, s, :], in0=acc3, scalar=QC[m][0],
                        in1=E3, op0=OP.add, op1=OP.mult,
                    )
                for m in range(K + 1):
                    s = K - m
                    nc.vector.tensor_tensor(
                        out=sn4[:, :, s, :], in0=sd4[:, :, s, :], in1=rep3,
                        op=OP.mult,
                    )

                def mv_slice(ser, cc, dlo, dn):
                    # moving AP iterating (d outer, s inner) over slot-major ser
                    v = ser[:].rearrange("p (c s d) -> p c d s", c=2, s=NS)
                    return v[:, cc, dlo : dlo + dn, :]

                # suffix matmuls + scans, per (half, kind); kind 0 = den first
                for h in range(2):
                    for kind in range(2):
                        ser = SERd if kind == 0 else SERn
                        outs = scd if kind == 0 else scn
                        dlo = QD * h
                        p0 = pb_ps.tile([128, QW], F32, tag="sx0", name="sx0")
                        p1 = pb_ps.tile([128, QW], F32, tag="sx1", name="sx1")
                        nc.tensor.matmul(
                            p0[:], mats[:, 128:256], mv_slice(ser, 0, dlo, QD),
                            start=True, stop=False,
                        )
                        nc.tensor.matmul(
                            p1[:], mats[:, 128:256], mv_slice(ser, 1, dlo, QD),
                            start=True, stop=True,
                        )
                        nc.tensor.matmul(
                            p0[:], mats[:, 256:384], mv_slice(ser, 1, dlo, QD),
                            start=False, stop=True,
                        )
                        for cc in range(2):
                            psrc = (p0 if cc == 0 else p1)[:]
                            if cc == 1:
                                # experiment: stage PSUM->SBUF f16 via ACT so
                                # the DVE scan can hit its 2x fp16 mode
                                stg = pb_sb.tile([128, QW], F16, tag="stg", name="stg")
                                nc.scalar.activation(stg[:], psrc, AF.Copy)
                                psrc = stg[:]
                            nc.vector.tensor_tensor_scan(
                                out=outs[cc][:, h * QW : (h + 1) * QW],
                                data0=bblk[:, cc * WB + h * QW : cc * WB + (h + 1) * QW],
                                data1=psrc,
                                initial=0.0, op0=OP.mult, op1=OP.add,
                            )

                # divides: attn = num * rcp(den + (den==0)) at slot K
                for h in range(2):
                    dlo = QD * h
                    for cc in range(2):
                        dv = scd[cc][:, h * QW : (h + 1) * QW].rearrange(
                            "p (d s) -> p d s", s=NS)[:, :, K : K + 1]
                        nv = scn[cc][:, h * QW : (h + 1) * QW].rearrange(
                            "p (d s) -> p d s", s=NS)[:, :, K : K + 1]
                        den0 = pb_sb.tile([128, QD], F32, tag="den0", name="den0")
                        nc.vector.scalar_tensor_tensor(
                            out=den0[:].unsqueeze(2), in0=dv, scalar=0.0, in1=dv,
                            op0=OP.is_equal, op1=OP.add,
                        )
                        rcp = pb_sb.tile([128, QD], F32, tag="rcp", name="rcp")
                        nc.vector.reciprocal(out=rcp[:], in_=den0[:])
                        nc.vector.tensor_tensor(
                            out=attn_nat[:, cc * HALF + dlo : cc * HALF + dlo + QD].unsqueeze(2),
                            in0=nv, in1=rcp[:].unsqueeze(2), op=OP.mult,
                        )

                # transpose attn to T layout (own half only)
                for dh in range(2):
                    for cc in range(2):
                        t75 = pc_ps.tile([QD, 128], F16, tag="t75", name="t75")
                        nc.tensor.transpose(
                            t75[:], attn_nat[:, cc * HALF + dh * QD : cc * HALF + dh * QD + QD],
                            mats[:, 0:128],
                        )
                        nc.scalar.activation(
                            a75[dh][:, cc * 128 : (cc + 1) * 128], t75[:], AF.Copy
                        )

                # phase-C Wf2 partial, then stage + ReduceScatter
                for i, (go, gn) in enumerate(GC):
                    nc.tensor.matmul(
                        pcp[i][:], Wf2m[0][:, go : go + gn], a75[0][:],
                        start=False, stop=False,
                    )
                    nc.tensor.matmul(
                        pcp[i][:], Wf2m[1][:, go : go + gn], a75[1][:],
                        start=False, stop=True,
                    )
                    nc.scalar.activation(pre16[i][:], pcp[i][:], AF.Copy)
                    nc.sync.dma_start(rs_in[go : go + gn, :], pre16[i][:])
                nc.gpsimd.collective_compute(
                    "ReduceScatter",
                    mybir.AluOpType.add,
                    replica_groups=[[0, 1], [2, 3], [4, 5], [6, 7]],
                    ins=[rs_in.opt()],
                    outs=[rs_out.opt()],
                )

                # post-collective tail: tanh-sigmoid blend in T layout.
                # diff/summ precomputed BEFORE the collective lands; both th
                # loads issued in parallel on separate queues.
                diff = [pc_sb.tile([QD, S], F16, tag=f"diff{i}", name=f"diff{i}") for i in range(2)]
                summ = [pc_sb.tile([QD, S], F16, tag=f"summ{i}", name=f"summ{i}") for i in range(2)]
                thi = [pc_sb.tile([QD, S], F16, tag=f"thi{i}", name=f"thi{i}") for i in range(2)]
                for dh in range(2):
                    nc.vector.tensor_tensor(
                        out=diff[dh][:], in0=repTm[dh][:], in1=a75[dh][:], op=OP.subtract
                    )
                    nc.vector.tensor_tensor(
                        out=summ[dh][:], in0=repTm[dh][:], in1=a75[dh][:], op=OP.add
                    )
                nc.sync.dma_start(thi[0][:], rs_out[0:QD, :])
                nc.scalar.dma_start(thi[1][:], rs_out[QD : 2 * QD, :])
                for dh in range(2):
                    nc.scalar.activation(th16[dh][:], thi[dh][:], AF.Tanh, scale=0.5)
                for dh in range(2):
                    nc.vector.tensor_tensor(
                        out=diff[dh][:], in0=th16[dh][:], in1=diff[dh][:], op=OP.mult
                    )
                    nc.vector.tensor_tensor(
                        out=summ[dh][:], in0=summ[dh][:], in1=diff[dh][:], op=OP.add
                    )
                    outt = pc_sb.tile([QD, S], F32, tag="outt", name="outt")
                    nc.vector.tensor_tensor(
                        out=outt[:], in0=summ[dh][:], in1=Mb[:], op=OP.mult
                    )
                    eng_o = nc.scalar if dh == 1 else nc.sync
                    eng_o.dma_start(outT_d[dh * QD : (dh + 1) * QD, :], outt[:])

    nc.compile()
    return nc


def _host_prep(inputs, rep_mask, W_fc, b_fc, W1, W2, b1, W_f1, W_f2, b_f):
    f = np.float32
    h = np.float16
    su = (np.arange(128)[:, None] > np.arange(128)[None, :]).astype(h)
    mats = np.concatenate(
        [np.eye(128, dtype=h), su, np.ones((128, 128), dtype=h)], axis=1
    )
    in_maps = []
    for c in range(8):
        b, hh = c // 2, c % 2
        if hh == 0:
            perm = np.arange(D)
        else:
            perm = np.concatenate([np.arange(HALF, D), np.arange(HALF)])
        myhalf = perm[:HALF]
        rm = rep_mask[b].astype(f)
        maskbias = np.stack(
            [(rm[0:128] - 1.0) * 30000.0, (rm[128:256] - 1.0) * 30000.0], axis=1
        ).astype(f)
        bf_eff = b_f if hh == 0 else np.zeros_like(b_f)
        crow = np.zeros(CROW_W, dtype=h)
        crow[O_ONES : O_ONES + S] = 1.0
        crow[O_BFC : O_BFC + D] = b_fc[perm].astype(h)
        crow[O_BFCH : O_BFCH + HALF] = b_fc[myhalf].astype(h)
        crow[O_B1H : O_B1H + HALF] = b1[myhalf].astype(h)
        crow[O_BF : O_BF + D] = bf_eff.astype(h)
        crow[O_HM : O_HM + S] = (0.5 * rm).astype(h)
        mp = {
            "inputsT": np.ascontiguousarray(inputs[b].T).astype(h),
            "W_fcT": np.ascontiguousarray(W_fc.T[:, perm]).astype(h),
            "W_fcTh": np.ascontiguousarray(W_fc.T[:, myhalf]).astype(h),
            "W1Th": np.ascontiguousarray(W1.T[perm][:, myhalf]).astype(h),
            "W2Th": np.ascontiguousarray(W2.T[perm][:, myhalf]).astype(h),
            "Wf1Tm": np.ascontiguousarray(W_f1.T[myhalf, :]).astype(h),
            "Wf2Tm": np.ascontiguousarray(W_f2.T[myhalf, :]).astype(h),
            "consts_row": crow.reshape(1, CROW_W),
            "mats": mats,
            "maskbias": maskbias,
            "bfc_col": b_fc[perm].astype(f).reshape(D, 1),
        }
        in_maps.append(mp)
    return in_maps


def _assemble(results):
    out = np.empty((B, S, D), dtype=np.float32)
    for b in range(B):
        full = np.concatenate(
            [results[2 * b]["outT"], results[2 * b + 1]["outT"]], axis=0
        )
        out[b] = full.T
    return out


def kernel(**inputs):
    from concourse.bass_utils import run_bass_kernel_spmd

    if "nc" not in _CACHE:
        _CACHE["nc"] = _build_nc()
    nc = _CACHE["nc"]

    in_maps = _host_prep(**inputs)
    res = run_bass_kernel_spmd(nc, in_maps, list(range(8)))
    return _assemble(res.results)


# revision 31
# speedup vs baseline: 1.0570x; 1.0570x over previous
"""DiSA (directional self-attention) Bass kernel for Trainium2, 8 cores.

Factorized algorithm (no [S,S,D] intermediate): with
  w = e^{a+b} * psi(a+b),  psi(x) = exp(C*tanh(x/C) - x),  a=dep, b=head,
approximate psi by a degree-K polynomial P (weighted LS fit on the actual
a+b range).  Taylor expansion P(a+b) = sum_m b^m Q_m(a) makes the softmax
separable; e^b cancels in the ratio:
  attn_res[i,d] = sum_m b_i^m N_m(i) / sum_m b_i^m D_m(i)
  D_m(i) = suffix_{j>i}[ mask_j e^{a_j} Q_m(a_j) ]          (x rep_j for N_m)

Sharding: SOLO per batch — core c computes batch c//2 fully (pairs
duplicate work).  No collectives at all: on this runtime the first
collective pays a ~46us channel-init floor plus ~10us peer-start stagger,
which dwarfs the duplicated compute.  Host reads cores 0,2,4,6.

Mapping per core (full d=300):
  - series: DVE Horner ladders (fp16, flat [128,600] = 256 s x 300 d),
    mask folded into e^a via ACT bias; num series = den series * rep (2x tt).
  - suffix sums over j: PE matmuls, strict-upper-tri / all-ones
    stationaries; moving AP (d outer, s inner) -> d-major PSUM, K=4 ->
    [128,375] per 75-d quarter = one PSUM bank.
  - sum_m b^m X_m: DVE tensor_tensor_scan (state = b*state + X_m) over
    d-major PSUM with a b-repeat tile (0 at slot 0 of each d-block).
  - gate: pre-act accumulated in PSUM (Wf1 part early, Wf2 x attn late),
    tanh-sigmoid blend in T layout, outT [300,256] f32.
"""

import numpy as np
from math import comb as _comb

B, S, D = 4, 256, 300
C = 5.0
K = 4                  # psi polynomial degree
NS = K + 1             # slots per d-plane
WB = D * NS            # 1500 cols per c-block (slot-major series)
QD = 75                # d-planes per suffix/scan quarter
QW = QD * NS           # 375 cols per suffix psum (single PSUM bank)
NQ = D // QD           # 4 quarters

# crow packed consts: [ones(256) | b_fc(300) | b1(300) | b_f(300) | 0.5*rm(256)]
O_ONES, O_BFC, O_B1, O_BF, O_HM = 0, 256, 556, 856, 1156
CROW_W = 1412

_CACHE: dict = {}


def _q_coeffs():
    # weighted LS fit of psi on the actual a+b range (Gaussian weight
    # matching the data distribution; measured range ~[-5.6, 4.9])
    lo, hi, sig = -5.8, 5.1, 2.0
    xs = np.linspace(lo, hi, 12001)
    psi = np.exp(C * np.tanh(xs / C) - xs)
    w = np.exp(-xs ** 2 / (2 * sig ** 2)) + 1e-3
    V = np.vander(xs, K + 1, increasing=True) * w[:, None]
    c, *_ = np.linalg.lstsq(V, psi * w, rcond=None)
    return [[float(c[m + j] * _comb(m + j, m)) for j in range(K - m + 1)]
            for m in range(K + 1)]


QC = _q_coeffs()


def _chunks(total, step=128):
    return [(s, min(step, total - s)) for s in range(0, total, step)]


def _build_nc():
    import concourse.bass as bass
    import concourse.tile as tile
    from concourse import bacc, mybir

    F32 = mybir.dt.float32
    F16 = mybir.dt.float16
    AF = mybir.ActivationFunctionType
    OP = mybir.AluOpType

    nc = bacc.Bacc("TRN2", target_bir_lowering=False, debug=False, num_devices=8)

    def din(name, shape, dt=F16):
        return nc.dram_tensor(name, shape, dt, kind="ExternalInput").ap()

    inputsT_d = din("inputsT", [D, S])
    W_fcT_d = din("W_fcT", [D, D])
    W1T_d = din("W1T", [D, D])
    W2T_d = din("W2T", [D, D])
    Wf1T_d = din("Wf1T", [D, D])
    Wf2T_d = din("Wf2T", [D, D])
    crow_d = din("consts_row", [1, CROW_W])
    mats_d = din("mats", [128, 384])        # [ident | su_tri | ones]
    maskb_d = din("maskbias", [128, 2], F32)
    bcol_d = din("bfc_col", [D, 1], F32)    # b_fc per-partition column
    outT_d = nc.dram_tensor("outT", [D, S], F32, kind="ExternalOutput").ap()

    DC = _chunks(D)
    GC = [(QD * q, QD) for q in range(NQ)]  # 75-row g-quarters for gate psum

    with tile.TileContext(nc) as tc:
        with tc.tile_pool(name="persist", bufs=1) as pp:
            # ---- input DMAs: critical tensors first, spread across queues
            inT = [pp.tile([n, S], F16, tag=f"inT{i}", name=f"inT{i}") for i, (o, n) in enumerate(DC)]
            WfcT = [pp.tile([n, D], F16, tag=f"wfc{i}", name=f"wfc{i}") for i, (o, n) in enumerate(DC)]
            W1T = [pp.tile([n, D], F16, tag=f"w1{i}", name=f"w1_{i}") for i, (o, n) in enumerate(DC)]
            W2T = [pp.tile([n, D], F16, tag=f"w2{i}", name=f"w2_{i}") for i, (o, n) in enumerate(DC)]
            Wf1m = [pp.tile([QD, D], F16, tag=f"wg1{i}", name=f"wg1_{i}") for i in range(NQ)]
            Wf2m = [pp.tile([QD, D], F16, tag=f"wg2{i}", name=f"wg2_{i}") for i in range(NQ)]
            crow = pp.tile([1, CROW_W], F16)
            mats = pp.tile([128, 384], F16)
            maskb = pp.tile([128, 2], F32)
            bcol = [pp.tile([n, 1], F32, tag=f"bc{i}", name=f"bc{i}") for i, (o, n) in enumerate(DC)]

            for i, (o, n) in enumerate(DC):
                nc.sync.dma_start(inT[i][:], inputsT_d[o : o + n, :])
                nc.scalar.dma_start(WfcT[i][:], W_fcT_d[o : o + n, :])
            nc.sync.dma_start(crow[:], crow_d[:])
            nc.scalar.dma_start(maskb[:], maskb_d[:])
            for i, (o, n) in enumerate(DC):
                nc.sync.dma_start(W1T[i][:], W1T_d[o : o + n, :])
                nc.scalar.dma_start(W2T[i][:], W2T_d[o : o + n, :])
                nc.gpsimd.dma_start(bcol[i][:], bcol_d[o : o + n, :])
            nc.gpsimd.dma_start(mats[:], mats_d[:])
            for i in range(NQ):
                nc.gpsimd.dma_start(Wf1m[i][:], Wf1T_d[i * QD : (i + 1) * QD, :])
                nc.gpsimd.dma_start(Wf2m[i][:], Wf2T_d[i * QD : (i + 1) * QD, :])

            repT = [pp.tile([n, S], F16, tag=f"repT{i}", name=f"repT{i}") for i, (o, n) in enumerate(DC)]
            repTm = [pp.tile([QD, S], F16, tag=f"rtm{i}", name=f"rtm{i}") for i in range(NQ)]
            rep_nat = pp.tile([128, 2 * D], F16)
            dep_nat = pp.tile([128, 2 * D], F16)
            head_nat = pp.tile([128, 2 * D], F16)
            E_t = pp.tile([128, 2 * D], F16)
            SERd = pp.tile([128, 2 * WB], F16)   # slot-major: col = c*WB + s*D + d
            SERn = pp.tile([128, 2 * WB], F16)
            bblk = pp.tile([128, 2 * WB], F16)   # d-major: col = c*WB + d*NS + s
            scd = [pp.tile([128, WB], F16, tag=f"scd{i}", name=f"scd{i}") for i in range(2)]
            scn = [pp.tile([128, WB], F16, tag=f"scn{i}", name=f"scn{i}") for i in range(2)]
            attn_nat = pp.tile([128, 2 * D], F16)
            a75 = [pp.tile([QD, S], F16, tag=f"a75_{i}", name=f"a75_{i}") for i in range(NQ)]
            th16 = [pp.tile([QD, S], F16, tag=f"th{i}", name=f"th{i}") for i in range(NQ)]
            Mb = pp.tile([QD, S], F16)

            # ---------- phase A ----------
            with (
                tc.tile_pool(name="pa_ps", bufs=2, space="PSUM") as pa_ps,
                tc.tile_pool(name="pa_sb", bufs=2) as pa_sb,
            ):
                def elu_from_psum(ps_ap, out_ap, n, bias=None):
                    # elu(x) = min(exp(x) - 1, relu(x)); optional per-partition
                    # bias column folded into both branches.
                    w = ps_ap.shape[1]
                    ex = pa_sb.tile([n, w], F16, tag="elu_e", name="elu_e")
                    nc.scalar.activation(
                        ex[:], ps_ap, AF.Exp,
                        bias=(bias if bias is not None else 0.0),
                    )
                    rl = pa_sb.tile([n, w], F16, tag="elu_r", name="elu_r")
                    if bias is None:
                        nc.vector.tensor_scalar(
                            out=rl[:], in0=ps_ap, scalar1=0.0, scalar2=None, op0=OP.max
                        )
                    else:
                        nc.vector.tensor_scalar(
                            out=rl[:], in0=ps_ap, scalar1=bias, scalar2=0.0,
                            op0=OP.add, op1=OP.max,
                        )
                    nc.vector.scalar_tensor_tensor(
                        out=out_ap, in0=ex[:], scalar=-1.0, in1=rl[:],
                        op0=OP.add, op1=OP.min,
                    )

                for i, (o, n) in enumerate(DC):
                    ps = pa_ps.tile([n, S], F32, tag="paT", name="paT")
                    for k in range(3):
                        nc.tensor.matmul(
                            ps[:], WfcT[k][:, o : o + n], inT[k][:],
                            start=(k == 0), stop=(k == 2),
                        )
                    elu_from_psum(ps[:], repT[i][:], n, bias=bcol[i][:])

                # Mb = broadcast of 0.5*rep_mask row (PE outer product)
                mps = pa_ps.tile([QD, S], F32, tag="mps", name="mps")
                nc.tensor.matmul(
                    mps[:], crow[0:1, O_ONES : O_ONES + QD],
                    crow[0:1, O_HM : O_HM + S],
                    start=True, stop=True,
                )
                nc.scalar.activation(Mb[:], mps[:], AF.Copy)

                # repTm copies (sbuf->sbuf DMA, 75-row quarters of repT)
                nc.sync.dma_start(repTm[0][:], repT[0][0:QD, :])
                nc.sync.dma_start(repTm[1][0:53, :], repT[0][QD:128, :])
                nc.scalar.dma_start(repTm[1][53:QD, :], repT[1][0:22, :])
                nc.scalar.dma_start(repTm[2][:], repT[1][22:97, :])
                nc.sync.dma_start(repTm[3][0:31, :], repT[1][97:128, :])
                nc.scalar.dma_start(repTm[3][31:QD, :], repT[2][0:44, :])

                for cc in range(2):
                    so = 128 * cc
                    ps = pa_ps.tile([128, D], F32, tag="paN", name="paN")
                    for k in range(3):
                        nc.tensor.matmul(
                            ps[:], inT[k][:, so : so + 128], WfcT[k][:],
                            start=(k == 0), stop=False,
                        )
                    # b_fc bias runs along the free (d) dim here: rank-1 add
                    nc.tensor.matmul(
                        ps[:], crow[0:1, O_ONES : O_ONES + 128],
                        crow[0:1, O_BFC : O_BFC + D],
                        start=False, stop=True,
                    )
                    elu_from_psum(ps[:], rep_nat[:, cc * D : (cc + 1) * D], 128)

                for cc in range(2):
                    so = 128 * cc
                    ps = pa_ps.tile([128, D], F32, tag="paN", name="paN")
                    for k in range(3):
                        nc.tensor.matmul(
                            ps[:], repT[k][:, so : so + 128], W1T[k][:],
                            start=(k == 0), stop=False,
                        )
                    nc.tensor.matmul(
                        ps[:], crow[0:1, O_ONES : O_ONES + 128],
                        crow[0:1, O_B1 : O_B1 + D],
                        start=False, stop=True,
                    )
                    # E = exp(dep + maskbias) straight from psum; dep copy on ACT
                    nc.scalar.activation(
                        E_t[:, cc * D : (cc + 1) * D], ps[:], AF.Exp,
                        bias=maskb[:, cc : cc + 1], scale=1.0,
                    )
                    nc.scalar.activation(
                        dep_nat[:, cc * D : (cc + 1) * D], ps[:], AF.Copy
                    )

                    ps2 = pa_ps.tile([128, D], F32, tag="paN", name="paN")
                    for k in range(3):
                        nc.tensor.matmul(
                            ps2[:], repT[k][:, so : so + 128], W2T[k][:],
                            start=(k == 0), stop=(k == 2),
                        )
                    nc.scalar.activation(
                        head_nat[:, cc * D : (cc + 1) * D], ps2[:], AF.Copy
                    )

            # ---------- phase B + C ----------
            with (
                tc.tile_pool(name="pb_sb", bufs=2) as pb_sb,
                tc.tile_pool(name="pb_ps", bufs=1, space="PSUM") as pb_ps,
                tc.tile_pool(name="tp_ps", bufs=1, space="PSUM") as tp_ps,
                tc.tile_pool(name="pc_ps", bufs=1, space="PSUM") as pc_ps,
                tc.tile_pool(name="pc_sb", bufs=2) as pc_sb,
            ):
                # gate Wf1 partial (early: PE stays warm during ladders)
                pcp = [pc_ps.tile([n, S], F32, tag=f"pcp{i}", name=f"pcp{i}") for i, (o, n) in enumerate(GC)]
                for i, (go, gn) in enumerate(GC):
                    for q in range(NQ):
                        nc.tensor.matmul(
                            pcp[i][:], Wf1m[q][:, go : go + gn], repTm[q][:],
                            start=(q == 0), stop=False,
                        )
                    nc.tensor.matmul(
                        pcp[i][:], crow[0:1, O_BF + go : O_BF + go + gn],
                        crow[0:1, O_ONES : O_ONES + S],
                        start=False, stop=False,
                    )

                # b-repeat tile (d-major): slot 0 -> 0, slots 1..K -> head (DVE)
                bb4 = bblk[:].rearrange("p (c d s) -> p c d s", c=2, s=NS)
                h3 = head_nat[:].rearrange("p (c d) -> p c d", c=2).unsqueeze(3)
                for s in range(1, NS):
                    nc.vector.tensor_scalar(
                        out=bb4[:, :, :, s : s + 1], in0=h3,
                        scalar1=0.0, scalar2=None, op0=OP.add,
                    )
                nc.vector.memset(bb4[:, :, :, 0:1], 0.0)

                # Horner ladders -> slot-major series (DVE).  den finals
                # first; num series = den series * rep (2x-mode tt).
                sd4 = SERd[:].rearrange("p (c s d) -> p c s d", c=2, s=NS)
                sn4 = SERn[:].rearrange("p (c s d) -> p c s d", c=2, s=NS)
                E3 = E_t[:].rearrange("p (c d) -> p c d", c=2)
                rep3 = rep_nat[:].rearrange("p (c d) -> p c d", c=2)

                for m in range(K + 1):
                    n = K - m
                    s = K - m
                    if n == 0:
                        nc.vector.tensor_scalar(
                            out=sd4[:, :, s, :], in0=E3,
                            scalar1=QC[m][0], scalar2=None, op0=OP.mult,
                        )
                        continue
                    acc = pb_sb.tile([128, 2 * D], F16, tag=f"acc{m}", name=f"acc{m}")
                    nc.vector.tensor_scalar(
                        out=acc[:], in0=dep_nat[:],
                        scalar1=QC[m][n], scalar2=None, op0=OP.mult,
                    )
                    for j in range(n - 1, 0, -1):
                        nc.vector.scalar_tensor_tensor(
                            out=acc[:], in0=acc[:], scalar=QC[m][j], in1=dep_nat[:],
                            op0=OP.add, op1=OP.mult,
                        )
                    acc3 = acc[:].rearrange("p (c d) -> p c d", c=2)
                    nc.vector.scalar_tensor_tensor(
                        out=sd4[:, :, s, :], in0=acc3, scalar=QC[m][0],
                        in1=E3, op0=OP.add, op1=OP.mult,
                    )
                for m in range(K + 1):
                    s = K - m
                    nc.vector.tensor_tensor(
                        out=sn4[:, :, s, :], in0=sd4[:, :, s, :], in1=rep3,
                        op=OP.mult,
                    )

                def mv_slice(ser, cc, dlo, dn):
                    # moving AP iterating (d outer, s inner) over slot-major ser
                    v = ser[:].rearrange("p (c s d) -> p c d s", c=2, s=NS)
                    return v[:, cc, dlo : dlo + dn, :]

                # suffix matmuls + scans per (quarter, kind); den first
                for q in range(NQ):
                    for kind in range(2):
                        ser = SERd if kind == 0 else SERn
                        outs = scd if kind == 0 else scn
                        dlo = QD * q
                        p0 = pb_ps.tile([128, QW], F32, tag="sx0", name="sx0")
                        p1 = pb_ps.tile([128, QW], F32, tag="sx1", name="sx1")
                        nc.tensor.matmul(
                            p0[:], mats[:, 128:256], mv_slice(ser, 0, dlo, QD),
                            start=True, stop=False,
                        )
                        nc.tensor.matmul(
                            p1[:], mats[:, 128:256], mv_slice(ser, 1, dlo, QD),
                            start=True, stop=True,
                        )
                        nc.tensor.matmul(
                            p0[:], mats[:, 256:384], mv_slice(ser, 1, dlo, QD),
                            start=False, stop=True,
                        )
                        for cc in range(2):
                            nc.vector.tensor_tensor_scan(
                                out=outs[cc][:, q * QW : (q + 1) * QW],
                                data0=bblk[:, cc * WB + q * QW : cc * WB + (q + 1) * QW],
                                data1=(p0 if cc == 0 else p1)[:],
                                initial=0.0, op0=OP.mult, op1=OP.add,
                            )

                # divides + transposes per quarter
                for q in range(NQ):
                    dlo = QD * q
                    for cc in range(2):
                        dv = scd[cc][:, q * QW : (q + 1) * QW].rearrange(
                            "p (d s) -> p d s", s=NS)[:, :, K : K + 1]
                        nv = scn[cc][:, q * QW : (q + 1) * QW].rearrange(
                            "p (d s) -> p d s", s=NS)[:, :, K : K + 1]
                        den0 = pb_sb.tile([128, QD], F32, tag="den0", name="den0")
                        nc.vector.scalar_tensor_tensor(
                            out=den0[:].unsqueeze(2), in0=dv, scalar=0.0, in1=dv,
                            op0=OP.is_equal, op1=OP.add,
                        )
                        rcp = pb_sb.tile([128, QD], F32, tag="rcp", name="rcp")
                        nc.vector.reciprocal(out=rcp[:], in_=den0[:])
                        nc.vector.tensor_tensor(
                            out=attn_nat[:, cc * D + dlo : cc * D + dlo + QD].unsqueeze(2),
                            in0=nv, in1=rcp[:].unsqueeze(2), op=OP.mult,
                        )
                    for cc in range(2):
                        t75 = tp_ps.tile([QD, 128], F16, tag="t75", name="t75")
                        nc.tensor.transpose(
                            t75[:], attn_nat[:, cc * D + dlo : cc * D + dlo + QD],
                            mats[:, 0:128],
                        )
                        nc.scalar.activation(
                            a75[q][:, cc * 128 : (cc + 1) * 128], t75[:], AF.Copy
                        )

                # gate Wf2 part, tanh, blend (all 75-row quarters, aligned)
                for i, (go, gn) in enumerate(GC):
                    for q in range(NQ):
                        nc.tensor.matmul(
                            pcp[i][:], Wf2m[q][:, go : go + gn], a75[q][:],
                            start=False, stop=(q == NQ - 1),
                        )
                    nc.scalar.activation(th16[i][:], pcp[i][:], AF.Tanh, scale=0.5)

                for q in range(NQ):
                    diff = pc_sb.tile([QD, S], F16, tag="diff", name="diff")
                    nc.vector.tensor_tensor(
                        out=diff[:], in0=repTm[q][:], in1=a75[q][:], op=OP.subtract
                    )
                    summ = pc_sb.tile([QD, S], F16, tag="summ", name="summ")
                    nc.vector.tensor_tensor(
                        out=summ[:], in0=repTm[q][:], in1=a75[q][:], op=OP.add
                    )
                    nc.vector.tensor_tensor(
                        out=diff[:], in0=th16[q][:], in1=diff[:], op=OP.mult
                    )
                    nc.vector.tensor_tensor(
                        out=summ[:], in0=summ[:], in1=diff[:], op=OP.add
                    )
                    outt = pc_sb.tile([QD, S], F32, tag="outt", name="outt")
                    nc.vector.tensor_tensor(
                        out=outt[:], in0=summ[:], in1=Mb[:], op=OP.mult
                    )
                    eng_o = nc.scalar if q % 2 == 1 else nc.sync
                    eng_o.dma_start(outT_d[q * QD : (q + 1) * QD, :], outt[:])

    nc.compile()
    return nc


def _host_prep(inputs, rep_mask, W_fc, b_fc, W1, W2, b1, W_f1, W_f2, b_f):
    f = np.float32
    h = np.float16
    su = (np.arange(128)[:, None] > np.arange(128)[None, :]).astype(h)
    mats = np.concatenate(
        [np.eye(128, dtype=h), su, np.ones((128, 128), dtype=h)], axis=1
    )
    in_maps = []
    for c in range(8):
        b = c // 2
        rm = rep_mask[b].astype(f)
        maskbias = np.stack(
            [(rm[0:128] - 1.0) * 30000.0, (rm[128:256] - 1.0) * 30000.0], axis=1
        ).astype(f)
        crow = np.zeros(CROW_W, dtype=h)
        crow[O_ONES : O_ONES + S] = 1.0
        crow[O_BFC : O_BFC + D] = b_fc.astype(h)
        crow[O_B1 : O_B1 + D] = b1.astype(h)
        crow[O_BF : O_BF + D] = b_f.astype(h)
        crow[O_HM : O_HM + S] = (0.5 * rm).astype(h)
        mp = {
            "inputsT": np.ascontiguousarray(inputs[b].T).astype(h),
            "W_fcT": np.ascontiguousarray(W_fc.T).astype(h),
            "W1T": np.ascontiguousarray(W1.T).astype(h),
            "W2T": np.ascontiguousarray(W2.T).astype(h),
            "Wf1T": np.ascontiguousarray(W_f1.T).astype(h),
            "Wf2T": np.ascontiguousarray(W_f2.T).astype(h),
            "consts_row": crow.reshape(1, CROW_W),
            "mats": mats,
            "maskbias": maskbias,
            "bfc_col": b_fc.astype(f).reshape(D, 1),
        }
        in_maps.append(mp)
    return in_maps


def _assemble(results):
    out = np.empty((B, S, D), dtype=np.float32)
    for b in range(B):
        out[b] = results[2 * b]["outT"].T
    return out


def kernel(**inputs):
    from concourse.bass_utils import run_bass_kernel_spmd

    if "nc" not in _CACHE:
        _CACHE["nc"] = _build_nc()
    nc = _CACHE["nc"]

    in_maps = _host_prep(**inputs)
    res = run_bass_kernel_spmd(nc, in_maps, list(range(8)))
    return _assemble(res.results)
